# revision 33
# baseline (speedup 1.0000x reference)
"""Trainium2 Bass kernel for windowed multi-lag autocorrelation.

Reference computation (per (batch, seq) row of x[16, 128, 8320]):
  - 64 overlapping windows of length 256, stride 128
  - per-window mean removal, hanning window
  - autocorrelation at lags 0..31, scaled by 1/256
  -> out [16, 128, 1, 64, 32]

Device formulation (quadratic op -> DFT trick so the PE does the work):
  autocorr(w)[a] = (1/N) sum_f alpha_f |DFT_N(w)|^2[f] * cos(2*pi*f*a/N)
  with N = 255 (odd -> rfft bins f=0..127 fill the 128 partitions exactly;
  no high-bin leftover block). N < 256+32 makes the transform circular, but
  the aliased lags 224..255 only touch hanning-damped window edges, adding
  ~1e-4 relative error (measured) against the 2e-2 gate.
  Mean removal + hanning fold into the forward matrix A [256, 256] =
  [cos f=0..127 | pad, sin f=1..127]; the pad column keeps sq_sin aligned
  so P[f] = C[f]^2 + S[f]^2 is a straight partition-aligned add.

  Everything runs in bf16 operands (fp32 PSUM accumulate): measured
  end-to-end rel err ~1.4e-3. bf16 halves input DMA (the memory floor)
  and SBUF footprint vs the fp32r baseline; PE rate is identical.

Per group of 8 rows (512 windows, free-dim column n = chunk*8 + row so
both window halves are stride-1 slices xv[:,0:512] / xv[:,8:520]):
  PE:    4 fwd matmuls (cos/sin x two 128-sample chunks, PSUM-accumulated)
         + 8 transposed inverse matmuls: stationary sq[:, 128k:128k+128]
         (cos^2 then sin^2, PSUM-accumulated so no explicit C^2+S^2 add),
         moving B [128f, 32lags] -> out [128 windows, 32 lags]. The
         transpose packs the output across all 128 partitions (a [32, ...]
         DMA would run 4x slower) and streams only 32 cols at full array
         utilization (~26ns each measured).
  Act:   sq_c = square(ps_cos) (PSUM -> bf16 SBUF; the activation engine
         is the only one that can square straight out of PSUM -- DVE
         tensor_tensor may read at most ONE input from PSUM, and Pool
         cannot touch PSUM at all). sq must stay bf16: walrus rejects
         mixed f32r/bf16 matmul operands (NCC_IBIR034) and an all-f32r
         inverse would run 4 cyc/row at free dim 32.
  DVE:   s_sb = copy(ps_sin) bf16; every 4th group copies ps_out to SBUF.
  Pool:  sq_s = s_sb * s_sb (~1025ns measured -- OK as its only op).
  The inverse of group g-2 issues after group g's forwards (lag-2 software
  pipelining, hiding the square's ~1us); 4 groups' inverse outputs share
  one [128, 512] PSUM bank.

Sharding: pure data parallel, 2 batches per core across 8 cores.
"""
import os

# must be set before NRT initializes: recovers cores left wedged by a
# previous crashed run (NRT_EXEC_UNIT_UNRECOVERABLE otherwise)
os.environ.setdefault("NEURON_RT_RESET_CORES", "1")

import numpy as np
import ml_dtypes

import concourse.bass as bass
import concourse.tile as tile
from concourse import mybir
from concourse.bass_utils import run_bass_kernel_spmd

NUM_AUTOCORR = 32
NUM_WINDOWS = 64
WIN_LEN = 256
WIN_STRIDE = 128
NFFT = 255
NF = 128  # rfft bins 0..127 (N odd)
SEQ = 128
BATCH = 16
VALUE = (NUM_WINDOWS - 1) * WIN_STRIDE + WIN_LEN  # 8320
NCHUNK = VALUE // WIN_STRIDE  # 65
N_CORES = 8
ROWS_PER_CORE = (BATCH // N_CORES) * SEQ  # 256
G = 8  # rows per group
NGROUP = ROWS_PER_CORE // G  # 32
NW = G * NUM_WINDOWS  # 512 windows per group (matmul free dim)
GW = G * NCHUNK  # 520 columns per group in the input tile
CCOL = 4 * 128 + NUM_AUTOCORR  # 544 const cols (Ac1|Ac2|As1|As2|B)
SB = 4  # groups stacked per output super-block (PSUM partition offsets)
NSB = NGROUP // SB  # 8
# progressive input DMA chunking: each dma_start costs ~565ns on the sync
# sequencer, so few big issues beat many small ones; chunk 0 additionally
# carries the consts so the PE can start after one transfer.
IN_CHUNKS = [1, 3, 8, 10, 10]
assert sum(IN_CHUNKS) == NGROUP

F32 = mybir.dt.float32
F32R = mybir.dt.float32r
BF16 = mybir.dt.bfloat16
N_WARMUP = 6  # dummy matmuls to ramp the PE clock while input DMA runs

LAST_EXEC_NS = None


def _build_mats():
    i = np.arange(WIN_LEN)
    f = np.arange(NF)
    h = np.hanning(WIN_LEN)
    ang = 2 * np.pi * np.outer(i, f) / NFFT
    C = h[:, None] * np.cos(ang)
    S = h[:, None] * np.sin(ang)
    Sb = np.zeros_like(S)
    Sb[:, 1:] = S[:, 1:]  # sin col j holds bin f=j; col 0 is a zero pad
    A = np.concatenate([C, Sb], axis=1)  # [256, 256]
    A = A - A.mean(axis=0, keepdims=True)  # fold per-window mean removal
    fa = 2 * np.pi * np.outer(f, np.arange(NUM_AUTOCORR)) / NFFT
    alpha = np.full(NF, 2.0)
    alpha[0] = 1.0
    B = alpha[:, None] * np.cos(fa) / (NFFT * WIN_LEN)
    return A.astype(np.float32), B.astype(np.float32)


def _split_sync_waits(nc, max_waits=1):
    """walrus in this container rejects instructions with multiple sem waits
    ("Too many sync wait commands"); split extras into single-wait NoOps."""
    ctr = [0]

    def mknop(engine, waits):
        ctr[0] += 1
        nop = mybir.InstNoOp(name=f"waitsplit-{ctr[0]}", ins=[], outs=[])
        nop.engine = engine
        nop.sync_info = mybir.SyncInfo(on_wait=list(waits), on_update=[])
        return nop

    for fn in nc.m.functions:
        for blk in fn.blocks:
            out = []
            changed = False
            for inst in blk.instructions:
                si = inst.sync_info
                waits = list(si.on_wait) if si is not None and si.on_wait else []
                if len(waits) > max_waits:
                    changed = True
                    extra, keep = waits[:-max_waits], waits[-max_waits:]
                    for k in range(0, len(extra), max_waits):
                        out.append(mknop(inst.engine, extra[k : k + max_waits]))
                    inst.sync_info = mybir.SyncInfo(
                        on_wait=keep, on_update=list(si.on_update or [])
                    )
                out.append(inst)
            if changed:
                blk.instructions = out
    return nc


def _build_kernel():
    nc = bass.Bass(target_bir_lowering=False)
    # xt[p, CCOL + g*520 + c*8 + r] = x[row 8g+r, 128c + p]; any column-range
    # DMA slice is per-partition contiguous in DRAM
    xt = nc.dram_tensor("xt", [128, CCOL + NGROUP * GW], BF16, kind="ExternalInput")
    out = nc.dram_tensor("out", [NSB, 128, NW], BF16, kind="ExternalOutput")

    with tile.TileContext(nc) as tc:
        with (
            tc.tile_pool(name="xin", bufs=1) as xpool,
            tc.tile_pool(name="sqp", bufs=4) as sqpool,
            tc.tile_pool(name="ssb", bufs=3) as spool,
            tc.tile_pool(name="outb", bufs=2) as opool,
            tc.tile_pool(name="psf", bufs=3, space="PSUM") as pspool,
            tc.tile_pool(name="pso", bufs=2, space="PSUM") as psopool,
        ):
            # PE p-state ramps 0.65 -> 2.4 GHz over ~3us of busy time; burn
            # the input-DMA wait on dummy matmuls over a memset tile so real
            # work starts at full clock
            warm = xpool.tile([128, 128 + NW], BF16, tag="warm")
            nc.gpsimd.memset(warm[:], 0)
            ps_warm = pspool.tile([128, 2 * NW], F32, tag="ps_cs")
            for _ in range(N_WARMUP):
                nc.tensor.matmul(
                    ps_warm[:, 0:NW], warm[:, 0:128], warm[:, 128 : 128 + NW],
                    start=True, stop=True,
                )

            # input in progressively-sized chunks; chunk 0 carries the consts
            chunk_tiles = []  # (tile, first_group, n_groups, col_offset)
            g0 = 0
            for ci, sz in enumerate(IN_CHUNKS):
                cols = sz * GW + (CCOL if ci == 0 else 0)
                xc_t = xpool.tile([128, cols], BF16, tag=f"xc{ci}")
                lo = 0 if ci == 0 else CCOL + g0 * GW
                nc.sync.dma_start(xc_t[:], xt.ap()[:, lo : lo + cols])
                chunk_tiles.append((xc_t, g0, sz, CCOL if ci == 0 else 0))
                g0 += sz

            c0 = chunk_tiles[0][0]
            a_c1 = c0[:, 0:128]
            a_c2 = c0[:, 128:256]
            a_s1 = c0[:, 256:384]
            a_s2 = c0[:, 384:512]
            b_w = c0[:, 512:544]

            def group_view(g):
                for t, gg0, sz, off in chunk_tiles:
                    if gg0 <= g < gg0 + sz:
                        lo = off + (g - gg0) * GW
                        return t[:, lo : lo + GW]
                raise AssertionError

            # lag-2 software pipeline: group g's inverse issues after group
            # g+2's forward, hiding the ~1us square from the PE
            pend = []  # [(g, sq), ...]
            psout_t = None

            def flush_inverse():
                nonlocal pend, psout_t
                if not pend:
                    return
                g, sq = pend.pop(0)
                j = g % SB
                if j == 0:
                    psout_t = psopool.tile([128, NW], F32, tag="ps_out")
                for k in range(4):
                    lo = 128 * j + 32 * k
                    nc.tensor.matmul(
                        psout_t[:, lo : lo + 32],
                        sq[:, 128 * k : 128 * k + 128], b_w,
                        start=True, stop=False,
                    )
                    nc.tensor.matmul(
                        psout_t[:, lo : lo + 32],
                        sq[:, NW + 128 * k : NW + 128 * k + 128], b_w,
                        start=False, stop=True,
                    )
                if j == SB - 1:
                    o_sb = opool.tile([128, NW], BF16, tag="o_sb")
                    nc.vector.tensor_copy(o_sb[:], psout_t[:])
                    # Act DGE queues: output doesn't FIFO behind bulk input
                    nc.scalar.dma_start(out.ap()[g // SB], o_sb[:])

            for g in range(NGROUP):
                xv = group_view(g)
                ps = pspool.tile([128, 2 * NW], F32, tag="ps_cs")
                nc.tensor.matmul(
                    ps[:, 0:NW], a_c1, xv[:, 0:NW], start=True, stop=False
                )
                nc.tensor.matmul(
                    ps[:, 0:NW], a_c2, xv[:, G : G + NW], start=False, stop=True
                )
                nc.tensor.matmul(
                    ps[:, NW : 2 * NW], a_s1, xv[:, 0:NW], start=True, stop=False
                )
                nc.tensor.matmul(
                    ps[:, NW : 2 * NW], a_s2, xv[:, G : G + NW],
                    start=False, stop=True,
                )

                if len(pend) >= 2:
                    flush_inverse()

                sq = sqpool.tile([128, 2 * NW], BF16, tag="sq")
                nc.scalar.square(sq[:, 0:NW], ps[:, 0:NW])
                s_sb = spool.tile([128, NW], BF16, tag="s_sb")
                nc.vector.tensor_copy(s_sb[:], ps[:, NW : 2 * NW])
                # all sin muls on Pool (~1050ns < group cadence): DVE keeps
                # only copy + outcopy, so output-copy groups no longer
                # overrun the cadence and stall the PE
                nc.gpsimd.tensor_mul(sq[:, NW : 2 * NW], s_sb[:], s_sb[:])
                pend.append((g, sq))

            while pend:
                flush_inverse()

    _split_sync_waits(nc)
    return nc


def _install_ntff_shim():
    """The trimmed antenv lacks axon_hooks, so trace=True degrades to no
    profile. Recreate the hook: ctypes into libaxon_pjrt.so (same ABI the
    boot shim uses), exposed as a synthetic antenv.axon_hooks module."""
    import sys
    import ctypes
    import contextlib
    import types

    if "antenv.axon_hooks" in sys.modules:
        return
    so_path = "/opt/axon/libaxon_pjrt.so"
    if not os.path.exists(so_path):
        return
    lib = ctypes.CDLL(so_path)
    if not hasattr(lib, "axon_start_nrt_profile"):
        return
    lib.axon_start_nrt_profile.argtypes = [
        ctypes.POINTER(ctypes.c_int64),
        ctypes.c_size_t,
    ]
    lib.axon_start_nrt_profile.restype = ctypes.c_int64
    lib.axon_stop_nrt_profile.argtypes = [ctypes.c_char_p]
    lib.axon_stop_nrt_profile.restype = ctypes.c_int64

    @contextlib.contextmanager
    def _hook(output_dir, device_ids):
        import jax

        jax.devices()
        if device_ids:
            ids = (ctypes.c_int64 * len(device_ids))(*device_ids)
            rc = lib.axon_start_nrt_profile(ids, len(device_ids))
        else:
            rc = lib.axon_start_nrt_profile(None, 0)
        if rc != 0:
            raise RuntimeError(f"axon_start_nrt_profile rc={rc}")
        try:
            yield
        finally:
            n = lib.axon_stop_nrt_profile(str(output_dir).encode())
            print(f"ntff profile: {n} file(s) -> {output_dir}")

    mod = types.ModuleType("antenv.axon_hooks")
    mod.get_axon_ntff_profile_hook = lambda: _hook
    mod.set_axon_ntff_profile_hook = lambda h: None
    sys.modules["antenv.axon_hooks"] = mod

    # avoid network-dependent artifact uploads in the trace path
    import concourse.bass_utils as bu

    bu.upload_artifacts = lambda tmpdir: f"local://{tmpdir}"


_NC_CACHE = None


def _get_nc():
    global _NC_CACHE
    if _NC_CACHE is None:
        _NC_CACHE = _build_kernel()
    return _NC_CACHE


def kernel(x: np.ndarray) -> np.ndarray:
    global LAST_EXEC_NS
    x = np.ascontiguousarray(np.asarray(x), dtype=np.float32)
    assert x.shape == (BATCH, SEQ, VALUE)

    A, B = _build_mats()
    consts = np.zeros((128, CCOL), np.float32)
    consts[:, 0:128] = A[0:128, 0:128]
    consts[:, 128:256] = A[128:256, 0:128]
    consts[:, 256:384] = A[0:128, 128:256]
    consts[:, 384:512] = A[128:256, 128:256]
    consts[:, 512:544] = B
    consts = consts.astype(ml_dtypes.bfloat16)

    bpc = BATCH // N_CORES
    in_maps = []
    for c in range(N_CORES):
        xc = x[c * bpc : (c + 1) * bpc]  # [2, 128, 8320]
        # xd[p, g, c, r] = x[row 8g+r, 128c + p]
        xd = (
            xc.reshape(NGROUP, G, NCHUNK, WIN_STRIDE)  # [g, r, c, p]
            .transpose(3, 0, 2, 1)  # [p, g, c, r]
            .reshape(128, NGROUP * GW)
            .astype(ml_dtypes.bfloat16)
        )
        xt = np.concatenate([consts, xd], axis=1)
        in_maps.append({"xt": np.ascontiguousarray(xt)})

    nc = _get_nc()
    trace = os.environ.get("AUTOCORR_TRACE", "0") == "1"
    if trace:
        _install_ntff_shim()
    try:
        res = run_bass_kernel_spmd(
            nc, in_maps, core_ids=list(range(N_CORES)), trace=trace
        )
    except Exception:
        # a stale/wedged device occasionally fails the first exec after a
        # fresh NEFF load; one retry has always recovered it
        res = run_bass_kernel_spmd(
            nc, in_maps, core_ids=list(range(N_CORES)), trace=trace
        )
    LAST_EXEC_NS = res.exec_time_ns

    outs = []
    for c in range(N_CORES):
        o = np.asarray(res.results[c]["out"]).astype(np.float32)
        # [sblock, partition p, col 128j+32k+a] where window-in-group
        # c*8+r = 128k+p, i.e. w = 16k + p//8, r = p%8, group = 4s+j
        o = o.reshape(NSB, 16, G, SB, 4, NUM_AUTOCORR)  # [s, pq, rp, j, k, a]
        o = o.transpose(0, 3, 2, 4, 1, 5)  # [s, j, rp, k, pq, a]
        outs.append(o.reshape(bpc, SEQ, NUM_WINDOWS, NUM_AUTOCORR))
    full = np.concatenate(outs, axis=0)  # [16, 128, 64, 32]
    return np.ascontiguousarray(full[:, :, None, :, :])


# revision 34
# speedup vs baseline: 1.0585x; 1.0585x over previous
"""Trainium2 Bass kernel for windowed multi-lag autocorrelation.

Reference computation (per (batch, seq) row of x[16, 128, 8320]):
  - 64 overlapping windows of length 256, stride 128
  - per-window mean removal, hanning window
  - autocorrelation at lags 0..31, scaled by 1/256
  -> out [16, 128, 1, 64, 32]

Device formulation (quadratic op -> DFT trick so the PE does the work):
  autocorr(w)[a] = (1/N) sum_f alpha_f |DFT_N(w)|^2[f] * cos(2*pi*f*a/N)
  with N = 255 (odd -> rfft bins f=0..127 fill the 128 partitions exactly;
  no high-bin leftover block). N < 256+32 makes the transform circular, but
  the aliased lags 224..255 only touch hanning-damped window edges, adding
  ~1e-4 relative error (measured) against the 2e-2 gate.
  Mean removal + hanning fold into the forward matrix A [256, 256] =
  [cos f=0..127 | pad, sin f=1..127]; the pad column keeps sq_sin aligned
  so P[f] = C[f]^2 + S[f]^2 is a straight partition-aligned add.

  Everything runs in bf16 operands (fp32 PSUM accumulate): measured
  end-to-end rel err ~1.4e-3. bf16 halves input DMA (the memory floor)
  and SBUF footprint vs the fp32r baseline; PE rate is identical.

Per group of 8 rows (512 windows, free-dim column n = chunk*8 + row so
both window halves are stride-1 slices xv[:,0:512] / xv[:,8:520]):
  PE:    4 fwd matmuls (cos/sin x two 128-sample chunks, PSUM-accumulated)
         + 8 transposed inverse matmuls: stationary sq[:, 128k:128k+128]
         (cos^2 then sin^2, PSUM-accumulated so no explicit C^2+S^2 add),
         moving B [128f, 32lags] -> out [128 windows, 32 lags]. The
         transpose packs the output across all 128 partitions (a [32, ...]
         DMA would run 4x slower) and streams only 32 cols at full array
         utilization (~26ns each measured).
  Act:   sq_c = square(ps_cos) (PSUM -> bf16 SBUF; the activation engine
         is the only one that can square straight out of PSUM -- DVE
         tensor_tensor may read at most ONE input from PSUM, and Pool
         cannot touch PSUM at all). sq must stay bf16: walrus rejects
         mixed f32r/bf16 matmul operands (NCC_IBIR034) and an all-f32r
         inverse would run 4 cyc/row at free dim 32.
  DVE:   s_sb = copy(ps_sin) bf16; every 4th group copies ps_out to SBUF.
  Pool:  sq_s = s_sb * s_sb (~1025ns measured -- OK as its only op).
  The inverse of group g-2 issues after group g's forwards (lag-2 software
  pipelining, hiding the square's ~1us); 4 groups' inverse outputs share
  one [128, 512] PSUM bank.

Sharding: pure data parallel, 2 batches per core across 8 cores.
"""
import os

# must be set before NRT initializes: recovers cores left wedged by a
# previous crashed run (NRT_EXEC_UNIT_UNRECOVERABLE otherwise)
os.environ.setdefault("NEURON_RT_RESET_CORES", "1")

import numpy as np
import ml_dtypes

import concourse.bass as bass
import concourse.tile as tile
from concourse import mybir
from concourse.bass_utils import run_bass_kernel_spmd

NUM_AUTOCORR = 32
NUM_WINDOWS = 64
WIN_LEN = 256
WIN_STRIDE = 128
NFFT = 255
NF = 128  # rfft bins 0..127 (N odd)
SEQ = 128
BATCH = 16
VALUE = (NUM_WINDOWS - 1) * WIN_STRIDE + WIN_LEN  # 8320
NCHUNK = VALUE // WIN_STRIDE  # 65
N_CORES = 8
ROWS_PER_CORE = (BATCH // N_CORES) * SEQ  # 256
G = 8  # rows per group
NGROUP = ROWS_PER_CORE // G  # 32
NW = G * NUM_WINDOWS  # 512 windows per group (matmul free dim)
GW = G * NCHUNK  # 520 columns per group in the input tile
CCOL = 4 * 128 + NUM_AUTOCORR  # 544 const cols (Ac1|Ac2|As1|As2|B)
SB = 4  # groups stacked per output super-block (PSUM partition offsets)
NSB = NGROUP // SB  # 8
# progressive input DMA chunking: each dma_start costs ~565ns on the sync
# sequencer, so few big issues beat many small ones; chunk 0 additionally
# carries the consts so the PE can start after one transfer.
IN_CHUNKS = [1, 3, 8, 10, 10]
assert sum(IN_CHUNKS) == NGROUP

F32 = mybir.dt.float32
F32R = mybir.dt.float32r
BF16 = mybir.dt.bfloat16
N_WARMUP = 6  # dummy matmuls to ramp the PE clock while input DMA runs

LAST_EXEC_NS = None


def _build_mats():
    i = np.arange(WIN_LEN)
    f = np.arange(NF)
    h = np.hanning(WIN_LEN)
    ang = 2 * np.pi * np.outer(i, f) / NFFT
    C = h[:, None] * np.cos(ang)
    S = h[:, None] * np.sin(ang)
    Sb = np.zeros_like(S)
    Sb[:, 1:] = S[:, 1:]  # sin col j holds bin f=j; col 0 is a zero pad
    A = np.concatenate([C, Sb], axis=1)  # [256, 256]
    A = A - A.mean(axis=0, keepdims=True)  # fold per-window mean removal
    fa = 2 * np.pi * np.outer(f, np.arange(NUM_AUTOCORR)) / NFFT
    alpha = np.full(NF, 2.0)
    alpha[0] = 1.0
    B = alpha[:, None] * np.cos(fa) / (NFFT * WIN_LEN)
    return A.astype(np.float32), B.astype(np.float32)


def _split_sync_waits(nc, max_waits=1):
    """walrus in this container rejects instructions with multiple sem waits
    ("Too many sync wait commands"); split extras into single-wait NoOps."""
    ctr = [0]

    def mknop(engine, waits):
        ctr[0] += 1
        nop = mybir.InstNoOp(name=f"waitsplit-{ctr[0]}", ins=[], outs=[])
        nop.engine = engine
        nop.sync_info = mybir.SyncInfo(on_wait=list(waits), on_update=[])
        return nop

    for fn in nc.m.functions:
        for blk in fn.blocks:
            out = []
            changed = False
            for inst in blk.instructions:
                si = inst.sync_info
                waits = list(si.on_wait) if si is not None and si.on_wait else []
                if len(waits) > max_waits:
                    changed = True
                    extra, keep = waits[:-max_waits], waits[-max_waits:]
                    for k in range(0, len(extra), max_waits):
                        out.append(mknop(inst.engine, extra[k : k + max_waits]))
                    inst.sync_info = mybir.SyncInfo(
                        on_wait=keep, on_update=list(si.on_update or [])
                    )
                out.append(inst)
            if changed:
                blk.instructions = out
    return nc


def _build_kernel():
    nc = bass.Bass(target_bir_lowering=False)
    # xt[p, CCOL + g*520 + c*8 + r] = x[row 8g+r, 128c + p]; any column-range
    # DMA slice is per-partition contiguous in DRAM
    xt = nc.dram_tensor("xt", [128, CCOL + NGROUP * GW], BF16, kind="ExternalInput")
    out = nc.dram_tensor("out", [NSB, 128, NW], BF16, kind="ExternalOutput")

    with tile.TileContext(nc) as tc:
        with (
            tc.tile_pool(name="xin", bufs=1) as xpool,
            tc.tile_pool(name="sqp", bufs=4) as sqpool,
            tc.tile_pool(name="ssb", bufs=3) as spool,
            tc.tile_pool(name="outb", bufs=2) as opool,
            tc.tile_pool(name="psf", bufs=3, space="PSUM") as pspool,
            tc.tile_pool(name="pso", bufs=2, space="PSUM") as psopool,
        ):
            # PE p-state ramps 0.65 -> 2.4 GHz over ~3us of busy time; burn
            # the input-DMA wait on dummy matmuls over a memset tile so real
            # work starts at full clock
            warm = xpool.tile([128, 128 + NW], BF16, tag="warm")
            nc.gpsimd.memset(warm[:], 0)
            ps_warm = pspool.tile([128, 2 * NW], F32, tag="ps_cs")
            for _ in range(N_WARMUP):
                nc.tensor.matmul(
                    ps_warm[:, 0:NW], warm[:, 0:128], warm[:, 128 : 128 + NW],
                    start=True, stop=True,
                )

            # input in progressively-sized chunks; chunk 0 carries the consts
            chunk_tiles = []  # (tile, first_group, n_groups, col_offset)
            g0 = 0
            for ci, sz in enumerate(IN_CHUNKS):
                cols = sz * GW + (CCOL if ci == 0 else 0)
                xc_t = xpool.tile([128, cols], BF16, tag=f"xc{ci}")
                lo = 0 if ci == 0 else CCOL + g0 * GW
                nc.sync.dma_start(xc_t[:], xt.ap()[:, lo : lo + cols])
                chunk_tiles.append((xc_t, g0, sz, CCOL if ci == 0 else 0))
                g0 += sz

            c0 = chunk_tiles[0][0]
            a_c1 = c0[:, 0:128]
            a_c2 = c0[:, 128:256]
            a_s1 = c0[:, 256:384]
            a_s2 = c0[:, 384:512]
            b_w = c0[:, 512:544]

            def group_view(g):
                for t, gg0, sz, off in chunk_tiles:
                    if gg0 <= g < gg0 + sz:
                        lo = off + (g - gg0) * GW
                        return t[:, lo : lo + GW]
                raise AssertionError

            # lag-2 software pipeline: group g's inverse issues after group
            # g+2's forward, hiding the ~1us square from the PE
            pend = []  # [(g, sq), ...]
            psout_t = None

            def flush_inverse():
                nonlocal pend, psout_t
                if not pend:
                    return
                g, sq = pend.pop(0)
                j = g % SB
                if j == 0:
                    psout_t = psopool.tile([128, NW], F32, tag="ps_out")
                for k in range(4):
                    lo = 128 * j + 32 * k
                    nc.tensor.matmul(
                        psout_t[:, lo : lo + 32],
                        sq[:, 128 * k : 128 * k + 128], b_w,
                        start=True, stop=False,
                    )
                    nc.tensor.matmul(
                        psout_t[:, lo : lo + 32],
                        sq[:, NW + 128 * k : NW + 128 * k + 128], b_w,
                        start=False, stop=True,
                    )
                if j == SB - 1:
                    o_sb = opool.tile([128, NW], BF16, tag="o_sb")
                    nc.vector.tensor_copy(o_sb[:], psout_t[:])
                    # Act DGE queues: output doesn't FIFO behind bulk input
                    nc.scalar.dma_start(out.ap()[g // SB], o_sb[:])

            for g in range(NGROUP):
                xv = group_view(g)
                ps = pspool.tile([128, 2 * NW], F32, tag="ps_cs")
                nc.tensor.matmul(
                    ps[:, 0:NW], a_c1, xv[:, 0:NW], start=True, stop=False
                )
                nc.tensor.matmul(
                    ps[:, 0:NW], a_c2, xv[:, G : G + NW], start=False, stop=True
                )
                nc.tensor.matmul(
                    ps[:, NW : 2 * NW], a_s1, xv[:, 0:NW], start=True, stop=False
                )
                nc.tensor.matmul(
                    ps[:, NW : 2 * NW], a_s2, xv[:, G : G + NW],
                    start=False, stop=True,
                )

                if len(pend) >= 2:
                    flush_inverse()

                sq = sqpool.tile([128, 2 * NW], BF16, tag="sq")
                nc.scalar.square(sq[:, 0:NW], ps[:, 0:NW])
                s_sb = spool.tile([128, NW], BF16, tag="s_sb")
                nc.vector.tensor_copy(s_sb[:], ps[:, NW : 2 * NW])
                # alternate the sin square between Pool and DVE: all-Pool
                # measured slower (its copy->mul chain eats the lag-2 slack
                # every group), all-DVE overruns the group cadence
                mul_eng = nc.gpsimd if g % 2 == 0 else nc.vector
                mul_eng.tensor_mul(sq[:, NW : 2 * NW], s_sb[:], s_sb[:])
                pend.append((g, sq))

            while pend:
                flush_inverse()

    _split_sync_waits(nc)
    return nc


def _install_ntff_shim():
    """The trimmed antenv lacks axon_hooks, so trace=True degrades to no
    profile. Recreate the hook: ctypes into libaxon_pjrt.so (same ABI the
    boot shim uses), exposed as a synthetic antenv.axon_hooks module."""
    import sys
    import ctypes
    import contextlib
    import types

    if "antenv.axon_hooks" in sys.modules:
        return
    so_path = "/opt/axon/libaxon_pjrt.so"
    if not os.path.exists(so_path):
        return
    lib = ctypes.CDLL(so_path)
    if not hasattr(lib, "axon_start_nrt_profile"):
        return
    lib.axon_start_nrt_profile.argtypes = [
        ctypes.POINTER(ctypes.c_int64),
        ctypes.c_size_t,
    ]
    lib.axon_start_nrt_profile.restype = ctypes.c_int64
    lib.axon_stop_nrt_profile.argtypes = [ctypes.c_char_p]
    lib.axon_stop_nrt_profile.restype = ctypes.c_int64

    @contextlib.contextmanager
    def _hook(output_dir, device_ids):
        import jax

        jax.devices()
        if device_ids:
            ids = (ctypes.c_int64 * len(device_ids))(*device_ids)
            rc = lib.axon_start_nrt_profile(ids, len(device_ids))
        else:
            rc = lib.axon_start_nrt_profile(None, 0)
        if rc != 0:
            raise RuntimeError(f"axon_start_nrt_profile rc={rc}")
        try:
            yield
        finally:
            n = lib.axon_stop_nrt_profile(str(output_dir).encode())
            print(f"ntff profile: {n} file(s) -> {output_dir}")

    mod = types.ModuleType("antenv.axon_hooks")
    mod.get_axon_ntff_profile_hook = lambda: _hook
    mod.set_axon_ntff_profile_hook = lambda h: None
    sys.modules["antenv.axon_hooks"] = mod

    # avoid network-dependent artifact uploads in the trace path
    import concourse.bass_utils as bu

    bu.upload_artifacts = lambda tmpdir: f"local://{tmpdir}"


_NC_CACHE = None


def _get_nc():
    global _NC_CACHE
    if _NC_CACHE is None:
        _NC_CACHE = _build_kernel()
    return _NC_CACHE


def kernel(x: np.ndarray) -> np.ndarray:
    global LAST_EXEC_NS
    x = np.ascontiguousarray(np.asarray(x), dtype=np.float32)
    assert x.shape == (BATCH, SEQ, VALUE)

    A, B = _build_mats()
    consts = np.zeros((128, CCOL), np.float32)
    consts[:, 0:128] = A[0:128, 0:128]
    consts[:, 128:256] = A[128:256, 0:128]
    consts[:, 256:384] = A[0:128, 128:256]
    consts[:, 384:512] = A[128:256, 128:256]
    consts[:, 512:544] = B
    consts = consts.astype(ml_dtypes.bfloat16)

    bpc = BATCH // N_CORES
    in_maps = []
    for c in range(N_CORES):
        xc = x[c * bpc : (c + 1) * bpc]  # [2, 128, 8320]
        # xd[p, g, c, r] = x[row 8g+r, 128c + p]
        xd = (
            xc.reshape(NGROUP, G, NCHUNK, WIN_STRIDE)  # [g, r, c, p]
            .transpose(3, 0, 2, 1)  # [p, g, c, r]
            .reshape(128, NGROUP * GW)
            .astype(ml_dtypes.bfloat16)
        )
        xt = np.concatenate([consts, xd], axis=1)
        in_maps.append({"xt": np.ascontiguousarray(xt)})

    nc = _get_nc()
    trace = os.environ.get("AUTOCORR_TRACE", "0") == "1"
    if trace:
        _install_ntff_shim()
    try:
        res = run_bass_kernel_spmd(
            nc, in_maps, core_ids=list(range(N_CORES)), trace=trace
        )
    except Exception:
        # a stale/wedged device occasionally fails the first exec after a
        # fresh NEFF load; one retry has always recovered it
        res = run_bass_kernel_spmd(
            nc, in_maps, core_ids=list(range(N_CORES)), trace=trace
        )
    LAST_EXEC_NS = res.exec_time_ns

    outs = []
    for c in range(N_CORES):
        o = np.asarray(res.results[c]["out"]).astype(np.float32)
        # [sblock, partition p, col 128j+32k+a] where window-in-group
        # c*8+r = 128k+p, i.e. w = 16k + p//8, r = p%8, group = 4s+j
        o = o.reshape(NSB, 16, G, SB, 4, NUM_AUTOCORR)  # [s, pq, rp, j, k, a]
        o = o.transpose(0, 3, 2, 4, 1, 5)  # [s, j, rp, k, pq, a]
        outs.append(o.reshape(bpc, SEQ, NUM_WINDOWS, NUM_AUTOCORR))
    full = np.concatenate(outs, axis=0)  # [16, 128, 64, 32]
    return np.ascontiguousarray(full[:, :, None, :, :])
